# revision 1
# baseline (speedup 1.0000x reference)
"""Trainium2 Bass kernel for nn_A2EvULoss (EvU loss over [1M, 100] logits).

Data-parallel over 8 NeuronCores; each core streams its 125k-row shard once
from HBM (p-major layout: partition p holds rows p*976+c, giving 6.4KB
contiguous DMA bursts per partition).

Streaming phase (per 16-row-tile chunk), balanced across engines:
 - DVE: scalar_tensor_tensor packs the class index into the 7 low mantissa
   bits of each logit ((x & ~0x7F) | (99-c)); one reduce_max then yields both
   the row max (to 2^-17 relative) and the argmax; plus a reduce_sum of
   exp(x) for 8 of 16 tiles and per-chunk min/max strips of sumexp.
 - ScalarE: batched exp for the DVE-summed tiles, plus per-tile exp with
   hardware accumulator (accum_out) for the other 8 tiles' sumexp.
The pack+max of the last 12 chunks is deferred to fill the min/max
all-reduce's latency window.

Tail: all-reduce (max) of (max sumexp, -min sumexp) gives global umin/umax
(unc = C/(C+sumexp) is monotone); per-row weights m1/m0 * tanh terms go to a
bf16 [128, 977, 4] block; 22-column step masks (21 thresholds + all-ones
totals) are built in 16 chunks interleaved with PSUM-accumulated matmuls
(4 stat-columns per matmul into a block-diagonal [16, 88] PSUM); the [4, 22]
sums are all-reduced and every core computes the trapezoid AUC + -log
replicated; core 0's scalar is returned.
"""

import numpy as np

P = 128
C = 100
N_CORES = 8
N_TOTAL = 1_000_000
NPC = N_TOTAL // N_CORES          # 125000 rows per core
MCOLS = NPC // P                  # 976 main stat columns (p-major layout)
REM = NPC - P * MCOLS             # 72 remainder rows
COLS = MCOLS + 1                  # 977 stat columns (col 976 = remainder)
T = 16                            # row-tiles per streaming chunk
NB_DVE = 8                        # tiles per chunk whose sumexp uses DVE reduce
NCHUNKS = MCOLS // T              # 61
DEFER = 12                        # trailing chunks whose pack+max fills cc1 wait
NTH = 21
K = NTH + 1                       # 21 thresholds + 1 all-ones (totals) column
EPS = 1e-10

F32 = None  # filled lazily (mybir import kept inside functions)


def _build_nc():
    import bass_rust
    import concourse.bass as bass
    import concourse.bacc as bacc
    import concourse.tile as tile
    from concourse import mybir

    f32 = mybir.dt.float32
    i32 = mybir.dt.int32
    bf16 = mybir.dt.bfloat16
    Op = mybir.AluOpType

    nc = bacc.Bacc("TRN2", target_bir_lowering=False, debug=False,
                   num_devices=N_CORES)

    x_d = nc.dram_tensor("x", [NPC, C], f32, kind="ExternalInput")
    tgt_d = nc.dram_tensor("tgt", [P, COLS], i32, kind="ExternalInput")
    valid_d = nc.dram_tensor("valid", [P, COLS], f32, kind="ExternalInput")
    iotas_d = nc.dram_tensor("iotas", [P, T * C], i32, kind="ExternalInput")
    iotak_d = nc.dram_tensor("iotak", [P, K], f32, kind="ExternalInput")
    out_d = nc.dram_tensor("out", [1, 1], f32, kind="ExternalOutput")

    x_main = x_d.ap()[0:P * MCOLS, :].rearrange("(p c) f -> p (c f)", p=P)
    x_rem = x_d.ap()[P * MCOLS:NPC, :]                      # [72, 100]

    with tile.TileContext(nc) as tc:
        with (
            tc.tile_pool(name="stream", bufs=3) as stream,
            tc.tile_pool(name="defstream", bufs=DEFER) as defstream,
            tc.tile_pool(name="persist", bufs=1) as persist,
            tc.tile_pool(name="psum", bufs=1, space="PSUM") as psump,
            tc.tile_pool(name="dram", bufs=1, space="DRAM") as dram,
        ):
            # ---- constants ----
            iota_big = persist.tile([P, T * C], i32)     # 99 - class, per tile
            nc.sync.dma_start(iota_big[:], iotas_d.ap())
            iota_kf = persist.tile([P, K], f32)
            nc.gpsimd.dma_start(iota_kf[:], iotak_d.ap())
            and_hi = persist.tile([P, 1], i32)
            nc.vector.memset(and_hi[:], -128)            # 0xFFFFFF80
            and_lo = persist.tile([P, 1], i32)
            nc.vector.memset(and_lo[:], 127)             # 0x0000007F

            tgt_sb = persist.tile([P, COLS], i32)
            nc.sync.dma_start(tgt_sb[:], tgt_d.ap())
            valid_sb = persist.tile([P, COLS], f32, tag="scr2")
            nc.sync.dma_start(valid_sb[:], valid_d.ap())

            # ---- per-row stats ----
            pmax = persist.tile([P, COLS], f32)          # packed row max
            sumexp = persist.tile([P, COLS], f32)
            padb = persist.tile([P, 1], f32)
            se_rem = persist.tile([P, 2], f32)   # (min-in, max-in) of col 976

            # warm-up/sync collective: wakes the CC path and re-syncs core
            # skew mid-stream so the real all-reduces wait less
            warm_in = dram.tile([1, 2], f32)
            warm_out = dram.tile([1, 2], f32)

            # ---- phase 1: stream x, compute packed max + sumexp ----
            # pack+max of the last DEFER chunks runs after the min/max
            # collective is issued, filling its latency window
            deferred = []

            def pack_max(xt, sl, nt):
                packed = stream.tile([P, T * C], i32, tag="packed")
                nc.vector.scalar_tensor_tensor(
                    packed[:, 0:nt * C], xt[:, 0:nt * C].bitcast(i32),
                    and_hi[:], iota_big[:, 0:nt * C],
                    op0=Op.bitwise_and, op1=Op.bitwise_or)
                nc.vector.reduce_max(
                    pmax[:, sl],
                    packed[:, 0:nt * C].bitcast(f32).rearrange(
                        "p (t f) -> p t f", f=C),
                    axis=mybir.AxisListType.X)

            # chunk 0 split into 4 small sub-chunks so compute starts as
            # soon as the first 200KB lands
            units = [(4 * i, 4, False) for i in range(4)]
            units += [(16 + T * i, T, False) for i in range(NCHUNKS - 1)]
            units += [(MCOLS, 1, True)]
            for ui, (c0u, nt, last) in enumerate(units):
                defer = ui >= len(units) - DEFER
                pool, tag = (defstream, "xtd") if defer else (stream, "xt")
                xt = pool.tile([P, T * C], f32, tag=tag)
                if last:
                    nc.vector.memset(xt[:, 0:C], 0.0)
                    nc.sync.dma_start(xt[0:REM, 0:C], x_rem)
                    sl = slice(MCOLS, COLS)
                else:
                    nc.sync.dma_start(
                        xt[:, 0:nt * C],
                        x_main[:, c0u * C:(c0u + nt) * C])
                    sl = slice(c0u, c0u + nt)

                ch = ui
                if defer:
                    deferred.append((xt, sl, nt))
                else:
                    pack_max(xt, sl, nt)
                if ui == 33:
                    nc.sync.dma_start(warm_in[:], iota_kf[0:1, 0:2])
                    nc.gpsimd.collective_compute(
                        "AllReduce", Op.max,
                        replica_groups=[list(range(N_CORES))],
                        ins=[warm_in[:].opt()], outs=[warm_out[:].opt()])

                # sumexp: split between DVE (batched exp + reduce) and ACT
                # (per-tile exp with accumulator) to balance the two engines
                nb = max(1, (nt * NB_DVE) // T) if nt > 1 else 1
                y = stream.tile([P, T * C], f32, tag="y")
                nc.scalar.activation(
                    y[:, 0:nb * C], xt[:, 0:nb * C],
                    mybir.ActivationFunctionType.Exp)
                nc.vector.reduce_sum(
                    sumexp[:, sl.start:sl.start + nb],
                    y[:, 0:nb * C].rearrange("p (t f) -> p t f", f=C),
                    axis=mybir.AxisListType.X)
                for t in range(nb, nt):
                    nc.scalar.activation(
                        y[:, t * C:(t + 1) * C], xt[:, t * C:(t + 1) * C],
                        mybir.ActivationFunctionType.Exp,
                        accum_out=sumexp[:, sl.start + t:sl.start + t + 1])

                # remainder-column pad mask (min/max of sumexp is reduced
                # once after the stream; unc is monotone in sumalpha)
                if last:
                    nc.scalar.activation(padb[:], valid_sb[:, MCOLS:COLS],
                                         mybir.ActivationFunctionType.Copy,
                                         bias=1e9, scale=-1e9)
                    nc.vector.tensor_add(se_rem[:, 0:1],
                                         sumexp[:, MCOLS:COLS], padb[:])
                    nc.vector.tensor_sub(se_rem[:, 1:2],
                                         sumexp[:, MCOLS:COLS], padb[:])

            # ---- phase 1b: per-row derived quantities ----
            c100 = persist.tile([P, 1], f32)
            nc.vector.memset(c100[:], float(C))
            c1 = persist.tile([P, 1], f32)
            nc.vector.memset(c1[:], 1.0)

            # all-reduce (max) of (max_sumexp, -min_sumexp); cross-partition
            # reduce stays on gpsimd so no DVE hop delays the issue
            mm = persist.tile([P, 2], f32)
            mhi = persist.tile([P, 1], f32)
            nc.vector.reduce_max(mhi[:], sumexp[:, 0:MCOLS],
                                 axis=mybir.AxisListType.X)
            nc.vector.tensor_tensor(mm[:, 0:1], mhi[:], se_rem[:, 1:2],
                                    op=Op.max)
            run_lo = persist.tile([P, 1], f32)
            nc.vector.tensor_reduce(run_lo[:], sumexp[:, 0:MCOLS],
                                    axis=mybir.AxisListType.X, op=Op.min)
            nc.vector.tensor_tensor(run_lo[:], run_lo[:], se_rem[:, 0:1],
                                    op=Op.min)
            nc.vector.tensor_scalar(mm[:, 1:2], run_lo[:], -1.0, None,
                                    Op.mult)
            mmr = persist.tile([P, 2], f32)
            nc.gpsimd.partition_all_reduce(mmr[:], mm[:], channels=P,
                                           reduce_op=bass_rust.ReduceOp.max)
            cc1_in = dram.tile([1, 2], f32)
            cc1_out = dram.tile([1, 2], f32)
            nc.sync.dma_start(cc1_in[:], mmr[0:1, :])
            nc.gpsimd.collective_compute(
                "AllReduce", Op.max,
                replica_groups=[list(range(N_CORES))],
                ins=[cc1_in[:].opt()], outs=[cc1_out[:].opt()])
            gmm = persist.tile([P, 2], f32)
            nc.sync.dma_start(
                gmm[:],
                bass.AP(tensor=cc1_out.tensor, offset=cc1_out[:].offset,
                        ap=[[0, P], [1, 2]]))

            # deferred pack+max fills the collective's latency window
            for xt_, sl_, nt_ in deferred:
                pack_max(xt_, sl_, nt_)

            # unc = C / (sumexp + C)
            sumalpha = persist.tile([P, COLS], f32)
            nc.scalar.activation(sumalpha[:], sumexp[:],
                                 mybir.ActivationFunctionType.Identity,
                                 bias=c100[:])
            rcp = persist.tile([P, COLS], f32)
            nc.vector.reciprocal(rcp[:], sumalpha[:])
            unc = persist.tile([P, COLS], f32)
            nc.scalar.mul(unc[:], rcp[:], float(C))

            # weights (independent of the collective; overlaps with it)
            rclean = persist.tile([P, COLS], f32, tag="scr1")
            nc.vector.tensor_scalar(rclean[:].bitcast(i32),
                                    pmax[:].bitcast(i32), and_hi[:], None,
                                    Op.bitwise_and)
            e = persist.tile([P, COLS], f32)             # max evidence
            nc.scalar.activation(e[:], rclean[:],
                                 mybir.ActivationFunctionType.Exp)
            t_ = persist.tile([P, COLS], f32)
            nc.scalar.activation(t_[:], unc[:],
                                 mybir.ActivationFunctionType.Tanh)
            omt = persist.tile([P, COLS], f32)           # 1 - t
            nc.scalar.activation(omt[:], t_[:],
                                 mybir.ActivationFunctionType.Identity,
                                 bias=c1[:], scale=-1.0)

            idx_i = persist.tile([P, COLS], i32, tag="scr1")
            nc.vector.tensor_scalar(idx_i[:], pmax[:].bitcast(i32),
                                    and_lo[:], None, Op.bitwise_and)
            corr = persist.tile([P, COLS], f32)          # pad tgt=-1 -> 0
            nc.vector.tensor_tensor(corr[:], idx_i[:], tgt_sb[:],
                                    op=Op.is_equal)

            m1 = persist.tile([P, COLS], f32, tag="scr2")            # correct: max_alpha
            nc.vector.scalar_tensor_tensor(m1[:], e[:], 1.0, corr[:],
                                           op0=Op.add, op1=Op.mult)
            m0 = persist.tile([P, COLS], f32)            # incorrect: 1-max_a
            cmv = persist.tile([P, COLS], f32, tag="scr3")
            nc.vector.tensor_sub(cmv[:], corr[:], valid_sb[:])
            nc.vector.tensor_mul(m0[:], cmv[:], e[:])

            w4 = persist.tile([P, COLS, 4], bf16)
            nc.vector.tensor_mul(w4[:, :, 0], m1[:], omt[:])   # ac
            nc.vector.tensor_mul(w4[:, :, 1], m1[:], t_[:])    # au
            nc.vector.tensor_mul(w4[:, :, 2], m0[:], omt[:])   # ic
            nc.vector.tensor_mul(w4[:, :, 3], m0[:], t_[:])    # iu

            # gmm holds (max_se, -min_se); umax = C/(C+min_se),
            # umin = C/(C+max_se)
            gsa = persist.tile([P, 2], f32)   # (C+max_se, C+min_se)
            nc.vector.tensor_scalar(gsa[:, 0:1], gmm[:, 0:1], float(C), None,
                                    Op.add)
            nc.vector.tensor_scalar(gsa[:, 1:2], gmm[:, 1:2], -1.0, float(C),
                                    Op.mult, Op.add)
            gu = persist.tile([P, 2], f32)    # (umin, umax)
            nc.vector.reciprocal(gu[:], gsa[:])
            nc.vector.tensor_scalar(gu[:], gu[:], float(C), None, Op.mult)
            # bucket b = clamp((unc - umin) * 20 / (umax - umin), <= 20)
            rng = persist.tile([P, 1], f32)
            nc.vector.tensor_sub(rng[:], gu[:, 1:2], gu[:, 0:1])
            rrng = persist.tile([P, 1], f32)
            nc.vector.reciprocal(rrng[:], rng[:])
            s1 = persist.tile([P, 1], f32)
            nc.vector.tensor_scalar(s1[:], rrng[:], float(NTH - 1), None,
                                    Op.mult)
            b = persist.tile([P, COLS], f32, tag="scr2")
            nc.vector.scalar_tensor_tensor(
                b[:], unc[:], gu[:, 0:1], s1[:].broadcast_to((P, COLS)),
                op0=Op.subtract, op1=Op.mult)
            bc = persist.tile([P, COLS], f32, tag="scr3")
            nc.vector.tensor_scalar(bc[:], b[:], float(NTH - 1), None, Op.min)

            # mask[p, c, k] = bc[p, c] <= k, built in chunks so the PE can
            # start accumulating while later chunks are still being built.
            # Matmuls run 4 stat-columns at a time into a block-diagonal
            # [16, 4K] PSUM; the 4 diagonal blocks are summed at the end.
            G = 4
            S = psump.tile([4 * G, G * K], f32)
            MCH = 16
            GCH = -(-COLS // (MCH * G)) * G              # columns per chunk
    
            with tc.tile_pool(name="maskp", bufs=2) as maskp:
                ngroups = -(-COLS // G)
                first, lastg = True, None
                for mi in range(MCH):
                    c0 = mi * GCH
                    c1e = min(c0 + GCH, COLS)
                    if c0 >= c1e:
                        continue
                    w = c1e - c0
                    mask = maskp.tile([P, GCH, K], bf16, tag="mask")
                    nc.vector.tensor_tensor(
                        mask[:, 0:w, :],
                        bc[:, c0:c1e].unsqueeze(2).broadcast_to((P, w, K)),
                        iota_kf[:].unsqueeze(1).broadcast_to((P, w, K)),
                        op=Op.is_le)
                    for g0 in range(c0, c1e, G):
                        gw = min(G, c1e - g0)
                        nc.tensor.matmul(
                            S[0:4 * gw, 0:gw * K],
                            w4[:, g0:g0 + gw, :],
                            mask[:, g0 - c0:g0 - c0 + gw, :],
                            start=(g0 == 0), stop=(g0 + gw >= COLS))

            s_all = persist.tile([4 * G, G * K], f32)
            nc.vector.tensor_copy(s_all[:], S[:])
            s_cat = persist.tile([4, G, K], f32)
            for g in range(G):
                nc.sync.dma_start(s_cat[:, g, :],
                                  s_all[4 * g:4 * g + 4, g * K:(g + 1) * K])
            s_sb = persist.tile([4, K], f32)
            nc.vector.tensor_reduce(
                s_sb[:],
                bass.AP(tensor=s_cat.tensor, offset=s_cat[:].offset,
                        ap=[list(s_cat[:].ap[0]), [1, K], [K, G]]),
                axis=mybir.AxisListType.X, op=Op.add)
            cc2_in = dram.tile([4, K], f32)
            cc2_out = dram.tile([4, K], f32)
            nc.sync.dma_start(cc2_in[:], s_sb[:])
            nc.gpsimd.collective_compute(
                "AllReduce", Op.add,
                replica_groups=[list(range(N_CORES))],
                ins=[cc2_in[:].opt()], outs=[cc2_out[:].opt()])

            # flatten [4, K] -> [1, 4K] on partition 0
            f = persist.tile([1, 4 * K], f32)
            nc.sync.dma_start(
                f[:], cc2_out[:].rearrange("a b -> (a b)").unsqueeze(0))

            ac = f[:, 0:NTH]
            aup = f[:, K:K + NTH]
            t_au = f[:, K + NTH:K + NTH + 1]
            ic = f[:, 2 * K:2 * K + NTH]
            iup = f[:, 3 * K:3 * K + NTH]
            t_iu = f[:, 3 * K + NTH:3 * K + NTH + 1]

            nneg_iu = persist.tile([1, NTH], f32)        # -n_iu
            nc.vector.tensor_scalar(nneg_iu[:], iup, t_iu, None, Op.subtract)
            num = persist.tile([1, NTH], f32)            # n_ac + n_iu
            nc.vector.tensor_sub(num[:], ac, nneg_iu[:])
            nneg_au = persist.tile([1, NTH], f32)        # -n_au
            nc.vector.tensor_scalar(nneg_au[:], aup, t_au, None, Op.subtract)
            den = persist.tile([1, NTH], f32)
            nc.vector.tensor_sub(den[:], num[:], nneg_au[:])
            nc.vector.tensor_add(den[:], den[:], ic)
            nc.vector.tensor_scalar(den[:], den[:], EPS, None, Op.add)
            rden = persist.tile([1, NTH], f32)
            nc.vector.reciprocal(rden[:], den[:])
            evu = persist.tile([1, NTH], f32)
            nc.vector.tensor_mul(evu[:], num[:], rden[:])

            ssum = persist.tile([1, 1], f32)
            nc.vector.reduce_sum(ssum[:], evu[:], axis=mybir.AxisListType.X)
            edge = persist.tile([1, 1], f32)
            nc.vector.tensor_add(edge[:], evu[:, 0:1], evu[:, NTH - 1:NTH])
            nc.vector.tensor_scalar(edge[:], edge[:], 0.5, None, Op.mult)
            auc = persist.tile([1, 1], f32)
            nc.vector.tensor_sub(auc[:], ssum[:], edge[:])
            nc.vector.tensor_scalar(auc[:], auc[:], 1.0 / (2 * (NTH - 1)) * 2,
                                    None, Op.mult)
            eps_t = persist.tile([1, 1], f32)
            nc.vector.memset(eps_t[:], EPS)
            nll = persist.tile([1, 1], f32)
            nc.scalar.activation(nll[:], auc[:],
                                 mybir.ActivationFunctionType.Ln,
                                 bias=eps_t[:])
            res = persist.tile([1, 1], f32)
            nc.vector.tensor_scalar(res[:], nll[:], -1.0, None, Op.mult)
            nc.sync.dma_start(out_d.ap(), res[:])

    nc.compile()
    return nc


_NC = None


def _get_nc():
    global _NC
    if _NC is None:
        _NC = _build_nc()
    return _NC


_VALID = None


def _valid_mask():
    global _VALID
    if _VALID is None:
        v = np.ones((P, COLS), np.float32)
        v[:, MCOLS] = 0.0
        v[:REM, MCOLS] = 1.0
        _VALID = v
    return _VALID


_IOTAS = None
_IOTAK = None


def _iotas():
    global _IOTAS
    if _IOTAS is None:
        row = (99 - np.arange(C, dtype=np.int32))
        _IOTAS = np.broadcast_to(np.tile(row, T), (P, T * C)).copy()
    return _IOTAS


def _iotak():
    global _IOTAK
    if _IOTAK is None:
        _IOTAK = np.broadcast_to(np.arange(K, dtype=np.float32),
                                 (P, K)).copy()
    return _IOTAK


def _in_maps(output, target):
    output = np.ascontiguousarray(np.asarray(output, dtype=np.float32))
    target = np.asarray(target)
    maps = []
    v = _valid_mask()
    for i in range(N_CORES):
        xs = output[i * NPC:(i + 1) * NPC]
        ts = np.asarray(target[i * NPC:(i + 1) * NPC], dtype=np.int64)
        t99 = (99 - ts).astype(np.int32)
        tgt = np.empty((P, COLS), np.int32)
        tgt[:, :MCOLS] = t99[:P * MCOLS].reshape(P, MCOLS)
        tgt[:, MCOLS] = -1
        tgt[:REM, MCOLS] = t99[P * MCOLS:]
        maps.append({"x": xs, "tgt": tgt, "valid": v,
                     "iotas": _iotas(), "iotak": _iotak()})
    return maps


def run(output, target, trace=False):
    from concourse.bass_utils import run_bass_kernel_spmd
    nc = _get_nc()
    res = run_bass_kernel_spmd(nc, _in_maps(output, target),
                               core_ids=list(range(N_CORES)), trace=trace)
    val = np.float32(res.results[0]["out"][0, 0])
    return val, res


def kernel(output, target, num_classes):
    assert int(num_classes) == C
    val, _ = run(output, target)
    return np.array(val, dtype=np.float32)



# revision 5
# speedup vs baseline: 1.3752x; 1.3752x over previous
"""Trainium2 Bass kernel for nn_A2EvULoss (EvU loss over [1M, 100] logits).

Data-parallel over 8 NeuronCores; each core streams its 125k-row shard once
from HBM (p-major layout: partition p holds rows p*976+c, giving 6.4KB
contiguous DMA bursts per partition).

Streaming phase (per 16-row-tile chunk):
 - ScalarE: one batched exp over the whole chunk, output in bf16.
 - DVE: pairwise bf16 add (2x mode) + fp32 segmented reduce_sum gives
   per-row sumexp; a bf16 max chain (two 2x tensor_tensor max levels, the
   second using an overlapped split since max is idempotent, then a short
   reduce_max) gives the per-row max evidence exp(xmax) directly.
The max chains of the last DEFER chunks run after the min/max all-reduce
is issued, filling its latency window.

Correctness test uses a host-side gather: xt[i] = x[i, target[i]] is fed
as a second (tiny) input; the row is correct iff bf16(exp(xt)) equals the
bf16 max of exp(x) (identical spline + rounding path, so equality is
exact for true argmax rows; bf16-tie false positives are ~7e-5 of rows).

Tail: all-reduce (max) of (max sumexp, -min sumexp) gives global umin/umax
(unc = C/(C+sumexp) is monotone); per-row weights go to four bf16 planes
(s-major [128, 4, 980]); 22 threshold masks are built per column chunk with
4x-mode tensor_scalar is_le against a bf16 bucket index; 245 PSUM-accumulated
matmuls produce block-diagonal [16, 88] sums; the [4, 22] totals are
all-reduced and every core computes the trapezoid AUC + -log replicated.
"""

import numpy as np

P = 128
C = 100
H = C // 2                        # 50: pairwise split of the class dim
N_CORES = 8
N_TOTAL = 1_000_000
NPC = N_TOTAL // N_CORES          # 125000 rows per core
MCOLS = NPC // P                  # 976 main stat columns (p-major layout)
REM = NPC - P * MCOLS             # 72 remainder rows
COLS = MCOLS + 1                  # 977 stat columns (col 976 = remainder)
CP = 984                          # padded to a multiple of G=8 for matmuls
T = 16                            # row-tiles per streaming chunk
NCHUNKS = MCOLS // T              # 61
DEFER = 12                        # trailing chunks whose max-chain fills cc1 wait
NTH = 21
K = NTH + 1                       # 21 thresholds + 1 all-ones (totals) column
G = 8                             # stat columns per matmul group
CW = 328                          # mask-build chunk width (41 groups; 3*328=984)
EPS = 1e-10


def _build_nc():
    import bass_rust
    import concourse.bass as bass
    import concourse.bacc as bacc
    import concourse.tile as tile
    from concourse import mybir

    f32 = mybir.dt.float32
    bf16 = mybir.dt.bfloat16
    Op = mybir.AluOpType
    Act = mybir.ActivationFunctionType
    X = mybir.AxisListType.X

    nc = bacc.Bacc("TRN2", target_bir_lowering=False, debug=False,
                   num_devices=N_CORES)

    x_d = nc.dram_tensor("x", [NPC, C], f32, kind="ExternalInput")
    xt_d = nc.dram_tensor("xt", [P, CP], f32, kind="ExternalInput")
    valid_d = nc.dram_tensor("valid", [P, CP], f32, kind="ExternalInput")
    out_d = nc.dram_tensor("out", [1, 1], f32, kind="ExternalOutput")

    x_main = x_d.ap()[0:P * MCOLS, :].rearrange("(p c) f -> p (c f)", p=P)
    x_rem = x_d.ap()[P * MCOLS:NPC, :]                      # [72, 100]

    with tile.TileContext(nc) as tc:
        with (
            tc.tile_pool(name="stream", bufs=3) as stream,
            tc.tile_pool(name="ystream", bufs=3) as ystream,
            tc.tile_pool(name="defstream", bufs=DEFER) as defstream,
            tc.tile_pool(name="persist", bufs=1) as persist,
            tc.tile_pool(name="psum", bufs=1, space="PSUM") as psump,
            tc.tile_pool(name="dram", bufs=1, space="DRAM") as dram,
        ):
            # ---- persistent inputs / stats ----
            xt_sb = persist.tile([P, CP], f32)
            nc.sync.dma_start(xt_sb[:], xt_d.ap())
            valid_sb = persist.tile([P, CP], f32)
            nc.sync.dma_start(valid_sb[:], valid_d.ap())

            pmax = persist.tile([P, CP], bf16)     # per-row max of exp(x)
            sumexp = persist.tile([P, CP], f32)
            nc.vector.memset(pmax[:, COLS:CP], 0.0)
            nc.vector.memset(sumexp[:, COLS:CP], 0.0)
            padb = persist.tile([P, 1], f32)
            se_rem = persist.tile([P, 2], f32)  # (min-in, max-in) of col 976

            c100 = persist.tile([P, 1], f32)
            nc.vector.memset(c100[:], float(C))

            # warm-up/sync collective: wakes the CC path and re-syncs core
            # skew mid-stream so the real all-reduces wait less
            warm_in = dram.tile([1, 2], f32)
            warm_out = dram.tile([1, 2], f32)

            # ---- phase 1: stream x, compute y=exp(x) bf16, max + sumexp ----
            deferred = []

            def maxchain(y, sl, nt):
                yv = y[:, 0:nt * C].rearrange("p (t f) -> p t f", f=C)
                mx1 = stream.tile([P, T, H], bf16, tag="mx1")
                nc.vector.tensor_tensor(
                    mx1[:, 0:nt, :], yv[:, :, 0:H], yv[:, :, H:C], op=Op.max)
                # overlapped split (24:50 vs 0:26) keeps both operands
                # 4B-aligned; max is idempotent so the overlap is harmless
                mx2 = stream.tile([P, T, 26], bf16, tag="mx2")
                nc.vector.tensor_tensor(
                    mx2[:, 0:nt, :], mx1[:, 0:nt, 0:26], mx1[:, 0:nt, 24:H],
                    op=Op.max)
                nc.vector.reduce_max(pmax[:, sl], mx2[:, 0:nt, :], axis=X)

            # chunk 0 split into 4 small sub-chunks so compute starts as
            # soon as the first 200KB lands
            units = [(4 * i, 4, False) for i in range(4)]
            units += [(16 + T * i, T, False) for i in range(NCHUNKS - 1)]
            units += [(MCOLS, 1, True)]
            for ui, (c0u, nt, last) in enumerate(units):
                defer = ui >= len(units) - DEFER
                xtile = stream.tile([P, T * C], f32, tag="xt")
                if last:
                    nc.vector.memset(xtile[:, 0:C], 0.0)
                    nc.sync.dma_start(xtile[0:REM, 0:C], x_rem)
                    sl = slice(MCOLS, COLS)
                else:
                    nc.sync.dma_start(
                        xtile[:, 0:nt * C],
                        x_main[:, c0u * C:(c0u + nt) * C])
                    sl = slice(c0u, c0u + nt)

                pool, tag = (defstream, "yd") if defer else (ystream, "y")
                y = pool.tile([P, T * C], bf16, tag=tag)
                nc.scalar.activation(y[:, 0:nt * C], xtile[:, 0:nt * C],
                                     Act.Exp)

                yv = y[:, 0:nt * C].rearrange("p (t f) -> p t f", f=C)
                s1t = stream.tile([P, T, H], bf16, tag="s1")
                nc.vector.tensor_tensor(
                    s1t[:, 0:nt, :], yv[:, :, 0:H], yv[:, :, H:C], op=Op.add)
                nc.vector.reduce_sum(sumexp[:, sl], s1t[:, 0:nt, :], axis=X)

                if defer:
                    deferred.append((y, sl, nt))
                else:
                    maxchain(y, sl, nt)

                if ui == 33:
                    nc.sync.dma_start(warm_in[:], valid_sb[0:1, 0:2])
                    nc.gpsimd.collective_compute(
                        "AllReduce", Op.max,
                        replica_groups=[list(range(N_CORES))],
                        ins=[warm_in[:].opt()], outs=[warm_out[:].opt()])

                # remainder-column pad strips (unc is monotone in sumexp)
                if last:
                    nc.scalar.activation(padb[:], valid_sb[:, MCOLS:COLS],
                                         Act.Copy, bias=1e9, scale=-1e9)
                    nc.vector.tensor_add(se_rem[:, 0:1],
                                         sumexp[:, MCOLS:COLS], padb[:])
                    nc.vector.tensor_sub(se_rem[:, 1:2],
                                         sumexp[:, MCOLS:COLS], padb[:])

            # ---- phase 1b: global umin/umax collective ----
            mm = persist.tile([P, 2], f32)
            mhi = persist.tile([P, 1], f32)
            nc.vector.reduce_max(mhi[:], sumexp[:, 0:MCOLS], axis=X)
            nc.vector.tensor_tensor(mm[:, 0:1], mhi[:], se_rem[:, 1:2],
                                    op=Op.max)
            run_lo = persist.tile([P, 1], f32)
            nc.vector.tensor_reduce(run_lo[:], sumexp[:, 0:MCOLS],
                                    axis=X, op=Op.min)
            nc.vector.tensor_tensor(run_lo[:], run_lo[:], se_rem[:, 0:1],
                                    op=Op.min)
            nc.vector.tensor_scalar(mm[:, 1:2], run_lo[:], -1.0, None,
                                    Op.mult)
            mmr = persist.tile([P, 2], f32)
            nc.gpsimd.partition_all_reduce(mmr[:], mm[:], channels=P,
                                           reduce_op=bass_rust.ReduceOp.max)
            cc1_in = dram.tile([1, 2], f32)
            cc1_out = dram.tile([1, 2], f32)
            nc.sync.dma_start(cc1_in[:], mmr[0:1, :])
            nc.gpsimd.collective_compute(
                "AllReduce", Op.max,
                replica_groups=[list(range(N_CORES))],
                ins=[cc1_in[:].opt()], outs=[cc1_out[:].opt()])
            gmm = persist.tile([P, 2], f32)
            nc.sync.dma_start(
                gmm[:],
                bass.AP(tensor=cc1_out.tensor, offset=cc1_out[:].offset,
                        ap=[[0, P], [1, 2]]))

            # deferred max chains fill the collective's latency window
            for y_, sl_, nt_ in deferred:
                maxchain(y_, sl_, nt_)

            # ---- per-row weights (independent of the collective) ----
            yt = persist.tile([P, CP], bf16)
            nc.scalar.activation(yt[:], xt_sb[:], Act.Exp)
            corr = persist.tile([P, CP], bf16)
            nc.vector.tensor_tensor(corr[:], pmax[:], yt[:], op=Op.is_equal)
            valid_b = persist.tile([P, CP], bf16)
            nc.vector.tensor_copy(valid_b[:], valid_sb[:])

            m1 = persist.tile([P, CP], bf16)             # correct: max_alpha
            nc.vector.scalar_tensor_tensor(m1[:], pmax[:], 1.0, corr[:],
                                           op0=Op.add, op1=Op.mult)
            cmv = persist.tile([P, CP], bf16)
            nc.vector.tensor_sub(cmv[:], corr[:], valid_b[:])
            m0 = persist.tile([P, CP], bf16)             # incorrect: 1-max_a
            nc.vector.tensor_mul(m0[:], cmv[:], pmax[:])

            sumalpha = persist.tile([P, CP], f32)
            nc.scalar.activation(sumalpha[:], sumexp[:], Act.Identity,
                                 bias=c100[:])
            rcp = persist.tile([P, CP], f32)
            nc.vector.reciprocal(rcp[:], sumalpha[:])
            unc = persist.tile([P, CP], f32)
            nc.scalar.mul(unc[:], rcp[:], float(C))
            t_ = persist.tile([P, CP], bf16)
            nc.scalar.activation(t_[:], unc[:], Act.Tanh)
            omt = persist.tile([P, CP], bf16)            # 1 - t
            nc.vector.tensor_scalar(omt[:], t_[:], -1.0, 1.0, Op.mult, Op.add)

            w4 = persist.tile([P, CP, 4], bf16)          # c-major weights
            nc.vector.tensor_mul(w4[:, :, 0], m1[:], omt[:])   # ac
            nc.vector.tensor_mul(w4[:, :, 1], m1[:], t_[:])    # au
            nc.vector.tensor_mul(w4[:, :, 2], m0[:], omt[:])   # ic
            nc.vector.tensor_mul(w4[:, :, 3], m0[:], t_[:])    # iu
            nc.vector.memset(w4[:, COLS:CP, :], 0.0)

            # gmm holds (max_se, -min_se); umax = C/(C+min_se),
            # umin = C/(C+max_se)
            gsa = persist.tile([P, 2], f32)   # (C+max_se, C+min_se)
            nc.vector.tensor_scalar(gsa[:, 0:1], gmm[:, 0:1], float(C), None,
                                    Op.add)
            nc.vector.tensor_scalar(gsa[:, 1:2], gmm[:, 1:2], -1.0, float(C),
                                    Op.mult, Op.add)
            gu = persist.tile([P, 2], f32)    # (umin, umax)
            nc.vector.reciprocal(gu[:], gsa[:])
            nc.vector.tensor_scalar(gu[:], gu[:], float(C), None, Op.mult)
            # bucket b = clamp((unc - umin) * 20 / (umax - umin), <= 20)
            rng = persist.tile([P, 1], f32)
            nc.vector.tensor_sub(rng[:], gu[:, 1:2], gu[:, 0:1])
            rrng = persist.tile([P, 1], f32)
            nc.vector.reciprocal(rrng[:], rng[:])
            s1v = persist.tile([P, 1], f32)
            nc.vector.tensor_scalar(s1v[:], rrng[:], float(NTH - 1), None,
                                    Op.mult)
            b = persist.tile([P, CP], f32)
            nc.vector.scalar_tensor_tensor(
                b[:], unc[:], gu[:, 0:1], s1v[:].broadcast_to((P, CP)),
                op0=Op.subtract, op1=Op.mult)
            bc = persist.tile([P, CP], bf16)
            nc.vector.tensor_scalar(bc[:], b[:], float(NTH - 1), None, Op.min)

            # mask[p, g, k, c] = bc[p, g*G+c] <= k: group-major layout keeps
            # the per-k build in 4x mode (innermost [1, G] bf16 writes) while
            # each group's [K, G] block stays contiguous, so the matmul
            # moving AP collapses to one free dim. Matmuls accumulate into a
            # block-diagonal [4G, GK] PSUM: S[c*4+s, k*G+c] += w4*mask.
            S = psump.tile([4 * G, G * K], f32)
            with tc.tile_pool(name="maskp", bufs=2) as maskp:
                NG = CW // G
                for c0 in range(0, CP, CW):
                    mask = maskp.tile([P, NG, K, G], bf16, tag="mask")
                    bcv = bc[:, c0:c0 + CW].rearrange(
                        "p (g c) -> p g c", c=G)
                    for k in range(K):
                        nc.vector.tensor_scalar(
                            mask[:, :, k, :], bcv, float(k), None, Op.is_le)
                    for gi in range(NG):
                        g0 = c0 + gi * G
                        nc.tensor.matmul(
                            S[:],
                            w4[:, g0:g0 + G, :],
                            mask[:, gi, :, :],
                            start=(g0 == 0), stop=(g0 + G >= CP))

            s_all = persist.tile([4 * G, G * K], f32)
            nc.vector.tensor_copy(s_all[:], S[:])
            # diagonal extraction: n_part[s, c, k] = S[c*4+s, k*G+c]
            s_cat = persist.tile([4, G, K], f32)
            for g in range(G):
                src = s_all[4 * g:4 * g + 4, g:g + 1 + (K - 1) * G]
                nc.sync.dma_start(
                    s_cat[:, g, :],
                    bass.AP(tensor=src.tensor, offset=src.offset,
                            ap=[list(src.ap[0]), [G, K]]))
            s_sb = persist.tile([4, K], f32)
            nc.vector.tensor_reduce(
                s_sb[:],
                bass.AP(tensor=s_cat.tensor, offset=s_cat[:].offset,
                        ap=[list(s_cat[:].ap[0]), [1, K], [K, G]]),
                axis=X, op=Op.add)
            cc2_in = dram.tile([4, K], f32)
            cc2_out = dram.tile([4, K], f32)
            nc.sync.dma_start(cc2_in[:], s_sb[:])
            nc.gpsimd.collective_compute(
                "AllReduce", Op.add,
                replica_groups=[list(range(N_CORES))],
                ins=[cc2_in[:].opt()], outs=[cc2_out[:].opt()])

            # flatten [4, K] -> [1, 4K] on partition 0
            f = persist.tile([1, 4 * K], f32)
            nc.sync.dma_start(
                f[:], cc2_out[:].rearrange("a b -> (a b)").unsqueeze(0))

            ac = f[:, 0:NTH]
            aup = f[:, K:K + NTH]
            t_au = f[:, K + NTH:K + NTH + 1]
            ic = f[:, 2 * K:2 * K + NTH]
            iup = f[:, 3 * K:3 * K + NTH]
            t_iu = f[:, 3 * K + NTH:3 * K + NTH + 1]

            nneg_iu = persist.tile([1, NTH], f32)        # -n_iu
            nc.vector.tensor_scalar(nneg_iu[:], iup, t_iu, None, Op.subtract)
            num = persist.tile([1, NTH], f32)            # n_ac + n_iu
            nc.vector.tensor_sub(num[:], ac, nneg_iu[:])
            nneg_au = persist.tile([1, NTH], f32)        # -n_au
            nc.vector.tensor_scalar(nneg_au[:], aup, t_au, None, Op.subtract)
            den = persist.tile([1, NTH], f32)
            nc.vector.tensor_sub(den[:], num[:], nneg_au[:])
            nc.vector.tensor_add(den[:], den[:], ic)
            nc.vector.tensor_scalar(den[:], den[:], EPS, None, Op.add)
            rden = persist.tile([1, NTH], f32)
            nc.vector.reciprocal(rden[:], den[:])
            evu = persist.tile([1, NTH], f32)
            nc.vector.tensor_mul(evu[:], num[:], rden[:])

            ssum = persist.tile([1, 1], f32)
            nc.vector.reduce_sum(ssum[:], evu[:], axis=X)
            edge = persist.tile([1, 1], f32)
            nc.vector.tensor_add(edge[:], evu[:, 0:1], evu[:, NTH - 1:NTH])
            nc.vector.tensor_scalar(edge[:], edge[:], 0.5, None, Op.mult)
            auc = persist.tile([1, 1], f32)
            nc.vector.tensor_sub(auc[:], ssum[:], edge[:])
            nc.vector.tensor_scalar(auc[:], auc[:], 1.0 / (NTH - 1), None,
                                    Op.mult)
            eps_t = persist.tile([1, 1], f32)
            nc.vector.memset(eps_t[:], EPS)
            nll = persist.tile([1, 1], f32)
            nc.scalar.activation(nll[:], auc[:], Act.Ln, bias=eps_t[:])
            res = persist.tile([1, 1], f32)
            nc.vector.tensor_scalar(res[:], nll[:], -1.0, None, Op.mult)
            nc.sync.dma_start(out_d.ap(), res[:])

    nc.compile()
    return nc


_NC = None


def _get_nc():
    global _NC
    if _NC is None:
        _NC = _build_nc()
    return _NC


_VALID = None


def _valid_mask():
    global _VALID
    if _VALID is None:
        v = np.ones((P, CP), np.float32)
        v[:, MCOLS:] = 0.0
        v[:REM, MCOLS] = 1.0
        _VALID = v
    return _VALID


def _in_maps(output, target):
    output = np.ascontiguousarray(np.asarray(output, dtype=np.float32))
    tgt = np.asarray(target).astype(np.int64)
    xt_full = output[np.arange(output.shape[0]), tgt].astype(np.float32)
    v = _valid_mask()
    maps = []
    for i in range(N_CORES):
        xs = output[i * NPC:(i + 1) * NPC]
        xtc = xt_full[i * NPC:(i + 1) * NPC]
        xtm = np.full((P, CP), -1e30, np.float32)
        xtm[:, :MCOLS] = xtc[:P * MCOLS].reshape(P, MCOLS)
        xtm[:REM, MCOLS] = xtc[P * MCOLS:]
        maps.append({"x": xs, "xt": xtm, "valid": v})
    return maps


def run(output, target, trace=False):
    from concourse.bass_utils import run_bass_kernel_spmd
    nc = _get_nc()
    res = run_bass_kernel_spmd(nc, _in_maps(output, target),
                               core_ids=list(range(N_CORES)), trace=trace)
    val = np.float32(res.results[0]["out"][0, 0])
    return val, res


def kernel(output, target, num_classes):
    assert int(num_classes) == C
    val, _ = run(output, target)
    return np.array(val, dtype=np.float32)
